# revision 2
# baseline (speedup 1.0000x reference)
"""Exponential decay envelope kernel for Trainium2 (8 NeuronCores).

Computes env[b, n] = r_b**n for b in [0, 512), n in [0, 96000) where
r_b = 1 - 6.91 / (48 * (10 + 1990 * decay_b)).

The store stream to HBM (~368 GB/s effective per core) is the wall, so
the design minimizes bytes written and keeps the stream saturated from
program start to end:

  * Per-row precision split: row b stores fp16 for cols [0, T_b) and
    fp8 e4m3 for [T_b, 96000), with T_b ~ 0.21 * decay_samples_b
    (bucketed to 3000, floor 6000, cap 24000).  A fixed global split
    wastes fp16 bytes on fast-decaying rows; the per-row split gives
    every row the same relative fp8 error (L2 ~ 5.4e-3, absmax ~ 8e-3
    vs the 2e-2 gate).
  * Rows are globally sorted by decay_samples (descending) and dealt
    round-robin to the 8 cores, so every core sees the same width
    profile and one SPMD program serves all cores.  Within a core,
    local rank j (slowest first) owns partitions 2j, 2j+1 (col halves).
    Widths are shared across cores via the per-octet max, so DRAM/SBUF
    shapes are identical.
  * Everything derives on-chip from a 1500-col fp16 seed
    (seedx[p, q] = r^(h*hw+q)) via per-partition scalar multiplies:
    head sections on DVE (fp16, 2x perf mode), tail sections split
    across DVE + ACT + GpSimd (fp8 out; all three together barely
    match the stream rate).
  * Stores are staircase rectangles over the rank-sorted partitions
    ([0:PX_k) for head, [PY_k:128) for tail), issued on the Sync HWDGE
    queue in earliest-deadline order with per-producer semaphore gates.
  * The head seed section [0,1500) is written by a DRAM->DRAM copy of
    the seedx input during the input-load latency window, when the
    store stream has nothing else to do; all other output bytes are
    written once, straight from SBUF.

Sharding: pure data parallel over batch; core c owns the rows with
global decay-rank g where g % 8 == c.
"""

import sys
import os

for _p in ("/opt/trn_rl_repo", "/opt/trn_rl_repo/pypackages"):
    if os.path.isdir(_p) and _p not in sys.path:
        sys.path.insert(0, _p)

import numpy as np
import ml_dtypes

import concourse.bass as bass
import concourse.bacc as bacc
import concourse.mybir as mybir
from concourse.bass_utils import run_bass_kernel_spmd

B = 512            # batch rows
N = 96000          # samples per row
M = 8              # cores
R = B // M         # rows per core = 64
HALF = N // 2      # per-partition row-col span = 48000
SEED = 1500        # seed width == head section width
YSEC = 3000        # tail section width

C_T = 0.21         # fp16->fp8 boundary: T_b ~= C_T * decay_samples_b
T_BUCKET = 3000
T_FLOOR = 6000
T_CAP = 24000

# schedule model (rel. to program start, us) used to order stores /
# balance the three tail producers; tuned against the HW trace
EST = {
    "dve_start": 2.0,    # loads landed
    "seedy_op": 0.70,    # DVE fp16 1500-col op
    "x_op": 0.65,        # DVE fp16 1500-col section
    "dve_y": 2.05,       # DVE fp8 3000-col section
    "act_y": 3.50,       # ACT fp8 3000-col section
    "gp_y": 6.00,        # GpSimd fp8 3000-col section
    "slow_start": 0.15,  # sem hop after seedY for ACT/GP
}

_F32 = mybir.dt.float32
_F16 = mybir.dt.float16
_FP8 = mybir.dt.float8e4

_cached = {}


def _rates(decay):
    """f32 rate exactly as the reference computes it, plus f64 log."""
    d = np.asarray(decay, dtype=np.float32).reshape(B)
    decay_ms = np.float32(10.0) + np.float32(1990.0) * d
    ds = (decay_ms * np.float32(48000.0)) / np.float32(1000.0)
    rate = np.float32(1.0) - np.float32(6.91) / ds
    return ds.astype(np.float64), np.log(rate.astype(np.float64))


def _geometry(ds):
    """Shared-across-cores widths from the actual decay values."""
    order = np.argsort(-ds, kind="stable")          # slowest first
    ds_sorted = ds[order]
    octmax = ds_sorted[0::M][:R]                    # max ds of octet j
    T = np.ceil(C_T * octmax / T_BUCKET) * T_BUCKET
    T = np.clip(T, T_FLOOR, T_CAP).astype(np.int64)  # [R], multiple of 3000
    hw = T // 2                                     # head half-width, mult 1500
    tw = HALF - hw                                  # tail half-width, mult 1500
    hw_p = np.repeat(hw, 2)                         # [128] per partition
    tw_p = np.repeat(tw, 2)
    nx = int(hw.max()) // SEED                      # head sections (incl seed)
    ny = -(-int(tw.max()) // YSEC)                  # tail sections (ceil)
    # head store k (k=1..nx-1): partitions [0, PX_k)
    PX = [int(np.count_nonzero(hw_p > SEED * k)) for k in range(nx)]
    # tail store k (k=0..ny-1): partitions [PY_k, 128)
    PY = [int(np.argmax(tw_p > YSEC * k)) for k in range(ny)]
    return order, T, hw, tw, nx, ny, PX, PY


def _schedule(nx, ny):
    """Assign tail sections to engines (list scheduling by earliest
    completion) and produce the EDF store order.

    Returns (assign, store_order):
      assign[k] = (engine, op_idx_1based) for tail section k
      store_order = list of ("x", k) / ("y", k) sorted by est. readiness
    """
    seedy_done = EST["dve_start"] + 2 * EST["seedy_op"]
    x_done = [seedy_done + EST["x_op"] * k for k in range(1, nx)]  # X1..
    avail = {
        "v": seedy_done + EST["x_op"] * (nx - 1),
        "a": seedy_done + EST["slow_start"],
        "g": seedy_done + EST["slow_start"],
    }
    cost = {"v": EST["dve_y"], "a": EST["act_y"], "g": EST["gp_y"]}
    nops = {"v": 0, "a": 0, "g": 0}
    picks = []
    for _ in range(ny):
        eng = min(avail, key=lambda e: avail[e] + cost[e])
        avail[eng] += cost[eng]
        nops[eng] += 1
        picks.append((avail[eng], eng, nops[eng]))
    picks.sort()
    # tail section k is computed by the k-th completed pick
    assign = [(eng, idx) for (_t, eng, idx) in picks]
    stores = [(x_done[k - 1], ("x", k)) for k in range(1, nx)]
    stores += [(picks[k][0], ("y", k)) for k in range(ny)]
    stores.sort()
    return assign, [s for (_t, s) in stores]


def _build_bass(geom):
    order, T, hw, tw, nx, ny, PX, PY = geom
    hwmax, cy = nx * SEED, ny * YSEC
    KK = max(nx, 2 * ny - 1)   # coef cols 0..KK-1 hold r^(1500k)
    NC = KK + 2                # + the two seedY multipliers
    assign, store_order = _schedule(nx, ny)

    nc = bacc.Bacc("TRN2", target_bir_lowering=False, debug=False, num_devices=M)

    seedx_t = nc.dram_tensor("seedx", [128, SEED], _F16, kind="ExternalInput")
    coef_t = nc.dram_tensor("coef", [128, NC], _F32, kind="ExternalInput")
    outx_t = nc.dram_tensor("outx", [128, hwmax], _F16, kind="ExternalOutput")
    outy_t = nc.dram_tensor("outy", [128, cy], _FP8, kind="ExternalOutput")

    bigx = nc.alloc_sbuf_tensor("bigx", [128, hwmax], _F16)
    bigy = nc.alloc_sbuf_tensor("bigy", [128, cy], _FP8)
    seedy = nc.alloc_sbuf_tensor("seedy", [128, 2 * SEED], _F16)
    coef_s = nc.alloc_sbuf_tensor("coef_s", [128, NC], _F32)

    n_stores = 1 + (nx - 1) + ny   # D2D + head + tail

    with (
        nc.semaphore("l_sem") as l_sem,      # +16 seedx load done
        nc.semaphore("c_sem") as c_sem,      # +16 coef load done
        nc.semaphore("s_sem") as s_sem,      # +1 per seedY op
        nc.semaphore("v_sem") as v_sem,      # +1 per DVE section (X then Y)
        nc.semaphore("a_sem") as a_sem,      # +1 per ACT tail section
        nc.semaphore("g_sem") as g_sem,      # +1 per GpSimd tail section
        nc.semaphore("d_sem") as d_sem,      # +16 per store
        nc.Block() as block,
    ):
        # DVE Y-op index -> v_sem threshold offset
        dve_y_rank = {}
        r = nx - 1
        for k, (eng, idx) in enumerate(assign):
            if eng == "v":
                dve_y_rank[k] = r + idx

        @block.sync
        def _(sync):
            sync.dma_start(bigx.ap()[:, 0:SEED], seedx_t.ap()).then_inc(l_sem, 16)
            sync.dma_start(coef_s.ap(), coef_t.ap()).then_inc(c_sem, 16)
            # seed section -> output, DRAM->DRAM, in the load-latency window
            sync.dma_start(outx_t.ap()[:, 0:SEED], seedx_t.ap()).then_inc(d_sem, 16)
            for kind, k in store_order:
                if kind == "x":
                    sync.wait_ge(v_sem, k)
                    p = PX[k]
                    sync.dma_start(
                        outx_t.ap()[0:p, SEED * k : SEED * (k + 1)],
                        bigx.ap()[0:p, SEED * k : SEED * (k + 1)],
                    ).then_inc(d_sem, 16)
                else:
                    eng, idx = assign[k]
                    sem = {"v": v_sem, "a": a_sem, "g": g_sem}[eng]
                    tgt = dve_y_rank[k] if eng == "v" else idx
                    sync.wait_ge(sem, tgt)
                    p = PY[k]
                    sync.dma_start(
                        outy_t.ap()[p:128, YSEC * k : YSEC * (k + 1)],
                        bigy.ap()[p:128, YSEC * k : YSEC * (k + 1)],
                    ).then_inc(d_sem, 16)
            sync.wait_ge(d_sem, 16 * n_stores)

        @block.vector
        def _(vector):
            vector.wait_ge(l_sem, 16)
            vector.wait_ge(c_sem, 16)
            for u in range(2):
                vector.tensor_scalar_mul(
                    seedy.ap()[:, u * SEED : (u + 1) * SEED],
                    bigx.ap()[:, 0:SEED],
                    coef_s.ap()[:, KK + u : KK + u + 1],
                ).then_inc(s_sem, 1)
            for k in range(1, nx):
                vector.tensor_scalar_mul(
                    bigx.ap()[:, SEED * k : SEED * (k + 1)],
                    bigx.ap()[:, 0:SEED],
                    coef_s.ap()[:, k : k + 1],
                ).then_inc(v_sem, 1)
            for k, (eng, _idx) in enumerate(assign):
                if eng == "v":
                    vector.tensor_scalar_mul(
                        bigy.ap()[:, YSEC * k : YSEC * (k + 1)],
                        seedy.ap(),
                        coef_s.ap()[:, 2 * k : 2 * k + 1],
                    ).then_inc(v_sem, 1)

        @block.scalar
        def _(scalar):
            scalar.wait_ge(s_sem, 2)
            for k, (eng, _idx) in enumerate(assign):
                if eng == "a":
                    scalar.activation(
                        bigy.ap()[:, YSEC * k : YSEC * (k + 1)],
                        seedy.ap(),
                        mybir.ActivationFunctionType.Copy,
                        scale=coef_s.ap()[:, 2 * k : 2 * k + 1],
                    ).then_inc(a_sem, 1)

        @block.gpsimd
        def _(gpsimd):
            gpsimd.wait_ge(s_sem, 2)
            for k, (eng, _idx) in enumerate(assign):
                if eng == "g":
                    gpsimd.tensor_scalar_mul(
                        bigy.ap()[:, YSEC * k : YSEC * (k + 1)],
                        seedy.ap(),
                        coef_s.ap()[:, 2 * k : 2 * k + 1],
                    ).then_inc(g_sem, 1)

    nc.finalize()
    return nc


def _host_precompute(geom, lnr):
    """Per-core seedx (fp16) and coef (f32) from fp64 host math."""
    order, T, hw, tw, nx, ny, PX, PY = geom
    KK = max(nx, 2 * ny - 1)
    NC = KK + 2
    q = np.arange(SEED, dtype=np.float64)
    h_p = np.tile(np.float64([0.0, 1.0]), R)        # [128]
    hw_p = np.repeat(hw, 2).astype(np.float64)
    tw_p = np.repeat(tw, 2).astype(np.float64)
    T_p = np.repeat(T, 2).astype(np.float64)
    in_maps = []
    for c in range(M):
        rows = order[c::M][:R]                      # local rank j -> row
        ln_p = np.repeat(lnr[rows], 2)              # [128]
        seedx = np.exp((h_p * hw_p)[:, None] * ln_p[:, None] + q[None, :] * ln_p[:, None])
        coef = np.empty((128, NC), dtype=np.float64)
        for k in range(KK):
            coef[:, k] = np.exp(SEED * k * ln_p)
        for u in range(2):
            coef[:, KK + u] = np.exp(
                (T_p + h_p * tw_p + SEED * u - h_p * hw_p) * ln_p
            )
        in_maps.append(
            {
                "seedx": seedx.astype(np.float16),
                "coef": coef.astype(np.float32),
            }
        )
    return in_maps


def _run(decay, **spmd_kwargs):
    ds, lnr = _rates(decay)
    key = ds.tobytes()
    if _cached.get("key") != key:
        geom = _geometry(ds)
        _cached.update(key=key, geom=geom, nc=_build_bass(geom))
    geom = _cached["geom"]
    order, T, hw, tw, nx, ny, PX, PY = geom
    in_maps = _host_precompute(geom, lnr)
    res = run_bass_kernel_spmd(_cached["nc"], in_maps, list(range(M)), **spmd_kwargs)
    out = np.empty((B, N), dtype=np.float32)
    for c in range(M):
        ox = np.asarray(res.results[c]["outx"]).astype(np.float32)
        oy = np.asarray(res.results[c]["outy"]).astype(np.float32)
        rows = order[c::M][:R]
        for j in range(R):
            b = rows[j]
            w, t = int(hw[j]), int(tw[j])
            out[b, 0:w] = ox[2 * j, 0:w]
            out[b, w : 2 * w] = ox[2 * j + 1, 0:w]
            out[b, 2 * w : 2 * w + t] = oy[2 * j, 0:t]
            out[b, 2 * w + t : N] = oy[2 * j + 1, 0:t]
    return out, res


def kernel(num_samples, decay):
    assert int(num_samples) == N, f"kernel compiled for {N} samples"
    out, _ = _run(decay)
    return out


# revision 8
# speedup vs baseline: 4.5431x; 4.5431x over previous
"""Exponential decay envelope kernel for Trainium2 (8 NeuronCores).

Computes env[b, n] = r_b**n for b in [0, 512), n in [0, 96000) where
r_b = 1 - 6.91 / (48 * (10 + 1990 * decay_b)).

The store stream to HBM (~368 GB/s effective per core) is the wall, so
the design minimizes bytes written and keeps the stream saturated from
program start to end:

  * Per-row precision split: row b stores fp16 for cols [0, T_b) and
    fp8 e4m3 for [T_b, 96000), with T_b ~ 0.21 * decay_samples_b
    (bucketed to 3000, floor 6000, cap 24000).  A fixed global split
    wastes fp16 bytes on fast-decaying rows; the per-row split gives
    every row the same relative fp8 error (L2 ~ 5.4e-3, absmax ~ 8e-3
    vs the 2e-2 gate).
  * Rows are globally sorted by decay_samples (descending) and dealt
    round-robin to the 8 cores, so every core sees the same width
    profile and one SPMD program serves all cores.  Within a core,
    local rank j (slowest first) owns partitions 2j, 2j+1 (col halves).
    Widths are shared across cores via the per-octet max, so DRAM/SBUF
    shapes are identical.
  * Everything derives on-chip from a 1500-col fp16 seed
    (seedx[p, q] = r^(h*hw+q)) via per-partition scalar multiplies:
    head sections on DVE (fp16, 2x perf mode), tail sections split
    across DVE + ACT (fp8 out; together they just match the stream
    rate — GpSimd tensor ops measured 43us/section and stall the DVE,
    so Pool does no compute).
  * Stores are staircase rectangles over the rank-sorted partitions
    ([0:PX_k) for head, [PY_k:128) for tail), issued on the Sync HWDGE
    queue in earliest-deadline order with per-producer semaphore gates.
  * The head seed section [0,1500) is written by a DRAM->DRAM copy of
    the seedx input during the input-load latency window, when the
    store stream has nothing else to do; all other output bytes are
    written once, straight from SBUF.

Sharding: pure data parallel over batch; core c owns the rows with
global decay-rank g where g % 8 == c.
"""

import sys
import os

for _p in ("/opt/trn_rl_repo", "/opt/trn_rl_repo/pypackages"):
    if os.path.isdir(_p) and _p not in sys.path:
        sys.path.insert(0, _p)

import numpy as np
import ml_dtypes

import concourse.bass as bass
import concourse.bacc as bacc
import concourse.mybir as mybir
from concourse.bass_utils import run_bass_kernel_spmd

B = 512            # batch rows
N = 96000          # samples per row
M = 8              # cores
R = B // M         # rows per core = 64
HALF = N // 2      # per-partition row-col span = 48000
SEED = 1500        # seed width == head section width
YSEC = 3000        # tail section width

C_T = 0.21         # fp16->fp8 boundary: T_b ~= C_T * decay_samples_b
T_BUCKET = 3000
T_FLOOR = 6000
T_CAP = 24000

# schedule model (rel. to program start, us) used to order stores /
# balance the two tail producers; tuned against the HW trace.
# GpSimd is NOT a producer: a Pool tensor op on [128,3000] measured 43us
# AND stalled concurrent DVE ops to the same 43us (SBUF port thrash).
EST = {
    "dve_start": 3.3,    # loads landed (seedx 384KB via sync HWDGE)
    "seedy_op": 0.57,    # DVE fp16 1500-col op
    "x_op": 0.52,        # DVE fp16 1500-col section
    "dve_y": 1.70,       # DVE fp8 3000-col section
    "act_y": 2.80,       # ACT fp8 3000-col section
    "slow_start": 0.20,  # sem hop after seedY for ACT
}

_F32 = mybir.dt.float32
_F16 = mybir.dt.float16
_FP8 = mybir.dt.float8e4

_cached = {}


def _rates(decay):
    """f32 rate exactly as the reference computes it, plus f64 log."""
    d = np.asarray(decay, dtype=np.float32).reshape(B)
    decay_ms = np.float32(10.0) + np.float32(1990.0) * d
    ds = (decay_ms * np.float32(48000.0)) / np.float32(1000.0)
    rate = np.float32(1.0) - np.float32(6.91) / ds
    return ds.astype(np.float64), np.log(rate.astype(np.float64))


def _geometry(ds):
    """Shared-across-cores widths from the actual decay values."""
    order = np.argsort(-ds, kind="stable")          # slowest first
    ds_sorted = ds[order]
    octmax = ds_sorted[0::M][:R]                    # max ds of octet j
    T = np.ceil(C_T * octmax / T_BUCKET) * T_BUCKET
    T = np.clip(T, T_FLOOR, T_CAP).astype(np.int64)  # [R], multiple of 3000
    hw = T // 2                                     # head half-width, mult 1500
    tw = HALF - hw                                  # tail half-width, mult 1500
    hw_p = np.repeat(hw, 2)                         # [128] per partition
    tw_p = np.repeat(tw, 2)
    nx = int(hw.max()) // SEED                      # head sections (incl seed)
    ny = -(-int(tw.max()) // YSEC)                  # tail sections (ceil)
    # head store k (k=1..nx-1): partitions [0, PX_k)
    PX = [int(np.count_nonzero(hw_p > SEED * k)) for k in range(nx)]
    # tail store k (k=0..ny-1): partitions [PY_k, 128)
    PY = [int(np.argmax(tw_p > YSEC * k)) for k in range(ny)]
    return order, T, hw, tw, nx, ny, PX, PY


def _schedule(nx, ny):
    """Assign tail sections to engines (list scheduling by earliest
    completion) and produce the EDF store order.

    Returns (assign, store_order):
      assign[k] = (engine, op_idx_1based) for tail section k
      store_order = list of ("x", k) / ("y", k) sorted by est. readiness
    """
    seedy_done = EST["dve_start"] + 2 * EST["seedy_op"]
    x_done = [seedy_done + EST["x_op"] * k for k in range(1, nx)]  # X1..
    avail = {
        "v": seedy_done + EST["x_op"] * (nx - 1),
        "a": seedy_done + EST["slow_start"],
    }
    cost = {"v": EST["dve_y"], "a": EST["act_y"]}
    nops = {"v": 0, "a": 0}
    picks = []
    for _ in range(ny):
        eng = min(avail, key=lambda e: avail[e] + cost[e])
        avail[eng] += cost[eng]
        nops[eng] += 1
        picks.append((avail[eng], eng, nops[eng]))
    picks.sort()
    # tail section k is computed by the k-th completed pick
    assign = [(eng, idx) for (_t, eng, idx) in picks]
    stores = [(x_done[k - 1], ("x", k)) for k in range(1, nx)]
    stores += [(picks[k][0], ("y", k)) for k in range(ny)]
    stores.sort()
    return assign, [s for (_t, s) in stores]


def _build_bass(geom):
    order, T, hw, tw, nx, ny, PX, PY = geom
    hwmax, cy = nx * SEED, ny * YSEC
    KK = max(nx, 2 * ny - 1)   # coef cols 0..KK-1 hold r^(1500k)
    NC = KK + 2                # + the two seedY multipliers
    assign, store_order = _schedule(nx, ny)

    nc = bacc.Bacc("TRN2", target_bir_lowering=False, debug=False, num_devices=M)

    seedx_t = nc.dram_tensor("seedx", [128, SEED], _F16, kind="ExternalInput")
    coef_t = nc.dram_tensor("coef", [128, NC], _F32, kind="ExternalInput")
    outx_t = nc.dram_tensor("outx", [128, hwmax], _F16, kind="ExternalOutput")
    outy_t = nc.dram_tensor("outy", [128, cy], _FP8, kind="ExternalOutput")

    bigx = nc.alloc_sbuf_tensor("bigx", [128, hwmax], _F16)
    bigy = nc.alloc_sbuf_tensor("bigy", [128, cy], _FP8)
    seedy = nc.alloc_sbuf_tensor("seedy", [128, 2 * SEED], _F16)
    coef_s = nc.alloc_sbuf_tensor("coef_s", [128, NC], _F32)

    n_stores = 1 + (nx - 1) + ny   # D2D + head + tail

    with (
        nc.semaphore("l_sem") as l_sem,      # +16 seedx load done
        nc.semaphore("c_sem") as c_sem,      # +16 coef load done
        nc.semaphore("s_sem") as s_sem,      # +1 per seedY op
        nc.semaphore("v_sem") as v_sem,      # +1 per DVE section (X then Y)
        nc.semaphore("a_sem") as a_sem,      # +1 per ACT tail section
        nc.semaphore("d_sem") as d_sem,      # +16 per store
        nc.Block() as block,
    ):
        # DVE Y-op index -> v_sem threshold offset
        dve_y_rank = {}
        r = nx - 1
        for k, (eng, idx) in enumerate(assign):
            if eng == "v":
                dve_y_rank[k] = r + idx

        @block.sync
        def _(sync):
            sync.dma_start(bigx.ap()[:, 0:SEED], seedx_t.ap()).then_inc(l_sem, 16)
            sync.dma_start(coef_s.ap(), coef_t.ap()).then_inc(c_sem, 16)
            # seed section -> output, DRAM->DRAM, in the load-latency window
            sync.dma_start(outx_t.ap()[:, 0:SEED], seedx_t.ap()).then_inc(d_sem, 16)
            for kind, k in store_order:
                if kind == "x":
                    sync.wait_ge(v_sem, k)
                    p = PX[k]
                    sync.dma_start(
                        outx_t.ap()[0:p, SEED * k : SEED * (k + 1)],
                        bigx.ap()[0:p, SEED * k : SEED * (k + 1)],
                    ).then_inc(d_sem, 16)
                else:
                    eng, idx = assign[k]
                    sem = v_sem if eng == "v" else a_sem
                    tgt = dve_y_rank[k] if eng == "v" else idx
                    sync.wait_ge(sem, tgt)
                    p = PY[k]
                    sync.dma_start(
                        outy_t.ap()[p:128, YSEC * k : YSEC * (k + 1)],
                        bigy.ap()[p:128, YSEC * k : YSEC * (k + 1)],
                    ).then_inc(d_sem, 16)
            sync.wait_ge(d_sem, 16 * n_stores)

        @block.vector
        def _(vector):
            vector.wait_ge(l_sem, 16)
            vector.wait_ge(c_sem, 16)
            for u in range(2):
                vector.tensor_scalar_mul(
                    seedy.ap()[:, u * SEED : (u + 1) * SEED],
                    bigx.ap()[:, 0:SEED],
                    coef_s.ap()[:, KK + u : KK + u + 1],
                ).then_inc(s_sem, 1)
            for k in range(1, nx):
                vector.tensor_scalar_mul(
                    bigx.ap()[:, SEED * k : SEED * (k + 1)],
                    bigx.ap()[:, 0:SEED],
                    coef_s.ap()[:, k : k + 1],
                ).then_inc(v_sem, 1)
            for k, (eng, _idx) in enumerate(assign):
                if eng == "v":
                    vector.tensor_scalar_mul(
                        bigy.ap()[:, YSEC * k : YSEC * (k + 1)],
                        seedy.ap(),
                        coef_s.ap()[:, 2 * k : 2 * k + 1],
                    ).then_inc(v_sem, 1)

        @block.scalar
        def _(scalar):
            scalar.wait_ge(s_sem, 2)
            for k, (eng, _idx) in enumerate(assign):
                if eng == "a":
                    scalar.activation(
                        bigy.ap()[:, YSEC * k : YSEC * (k + 1)],
                        seedy.ap(),
                        mybir.ActivationFunctionType.Copy,
                        scale=coef_s.ap()[:, 2 * k : 2 * k + 1],
                    ).then_inc(a_sem, 1)

    nc.finalize()
    return nc


def _host_precompute(geom, lnr):
    """Per-core seedx (fp16) and coef (f32) from fp64 host math."""
    order, T, hw, tw, nx, ny, PX, PY = geom
    KK = max(nx, 2 * ny - 1)
    NC = KK + 2
    q = np.arange(SEED, dtype=np.float64)
    h_p = np.tile(np.float64([0.0, 1.0]), R)        # [128]
    hw_p = np.repeat(hw, 2).astype(np.float64)
    tw_p = np.repeat(tw, 2).astype(np.float64)
    T_p = np.repeat(T, 2).astype(np.float64)
    in_maps = []
    for c in range(M):
        rows = order[c::M][:R]                      # local rank j -> row
        ln_p = np.repeat(lnr[rows], 2)              # [128]
        seedx = np.exp((h_p * hw_p)[:, None] * ln_p[:, None] + q[None, :] * ln_p[:, None])
        coef = np.empty((128, NC), dtype=np.float64)
        for k in range(KK):
            coef[:, k] = np.exp(SEED * k * ln_p)
        for u in range(2):
            coef[:, KK + u] = np.exp(
                (T_p + h_p * tw_p + SEED * u - h_p * hw_p) * ln_p
            )
        in_maps.append(
            {
                "seedx": seedx.astype(np.float16),
                "coef": coef.astype(np.float32),
            }
        )
    return in_maps


def _run(decay, **spmd_kwargs):
    ds, lnr = _rates(decay)
    key = ds.tobytes()
    if _cached.get("key") != key:
        geom = _geometry(ds)
        _cached.update(key=key, geom=geom, nc=_build_bass(geom))
    geom = _cached["geom"]
    order, T, hw, tw, nx, ny, PX, PY = geom
    in_maps = _host_precompute(geom, lnr)
    res = run_bass_kernel_spmd(_cached["nc"], in_maps, list(range(M)), **spmd_kwargs)
    out = np.empty((B, N), dtype=np.float32)
    for c in range(M):
        ox = np.asarray(res.results[c]["outx"]).astype(np.float32)
        oy = np.asarray(res.results[c]["outy"]).astype(np.float32)
        rows = order[c::M][:R]
        for j in range(R):
            b = rows[j]
            w, t = int(hw[j]), int(tw[j])
            out[b, 0:w] = ox[2 * j, 0:w]
            out[b, w : 2 * w] = ox[2 * j + 1, 0:w]
            out[b, 2 * w : 2 * w + t] = oy[2 * j, 0:t]
            out[b, 2 * w + t : N] = oy[2 * j + 1, 0:t]
    return out, res


def kernel(num_samples, decay):
    assert int(num_samples) == N, f"kernel compiled for {N} samples"
    out, _ = _run(decay)
    return out


# revision 11
# speedup vs baseline: 4.5834x; 1.0089x over previous
"""Exponential decay envelope kernel for Trainium2 (8 NeuronCores).

Computes env[b, n] = r_b**n for b in [0, 512), n in [0, 96000) where
r_b = 1 - 6.91 / (48 * (10 + 1990 * decay_b)).

The store stream to HBM (~368 GB/s effective per core) is the wall, so
the design minimizes bytes written and keeps the stream saturated from
program start to end:

  * Per-row precision split: row b stores fp16 for cols [0, T_b) and
    fp8 e4m3 for [T_b, 96000), with T_b ~ 0.21 * decay_samples_b
    (bucketed to 3000, floor 6000, cap 24000).  A fixed global split
    wastes fp16 bytes on fast-decaying rows; the per-row split gives
    every row the same relative fp8 error (L2 ~ 5.4e-3, absmax ~ 8e-3
    vs the 2e-2 gate).
  * Rows are globally sorted by decay_samples (descending) and dealt
    round-robin to the 8 cores, so every core sees the same width
    profile and one SPMD program serves all cores.  Within a core,
    local rank j (slowest first) owns partitions 2j, 2j+1 (col halves).
    Widths are shared across cores via the per-octet max, so DRAM/SBUF
    shapes are identical.
  * Everything derives on-chip from a 1500-col fp16 seed
    (seedx[p, q] = r^(h*hw+q)) via per-partition scalar multiplies:
    head sections on DVE (fp16, 2x perf mode), tail sections split
    across DVE + ACT (fp8 out; together they just match the stream
    rate — GpSimd tensor ops measured 43us/section and stall the DVE,
    so Pool does no compute).
  * Stores are staircase rectangles over the rank-sorted partitions
    ([0:PX_k) for head, [PY_k:128) for tail), issued on the Sync HWDGE
    queue in earliest-deadline order with per-producer semaphore gates.
  * The head seed section [0,1500) is written by a DRAM->DRAM copy of
    the seedx input during the input-load latency window, when the
    store stream has nothing else to do; all other output bytes are
    written once, straight from SBUF.

Sharding: pure data parallel over batch; core c owns the rows with
global decay-rank g where g % 8 == c.
"""

import sys
import os

for _p in ("/opt/trn_rl_repo", "/opt/trn_rl_repo/pypackages"):
    if os.path.isdir(_p) and _p not in sys.path:
        sys.path.insert(0, _p)

import numpy as np
import ml_dtypes

import concourse.bass as bass
import concourse.bacc as bacc
import concourse.mybir as mybir
from concourse.bass_utils import run_bass_kernel_spmd

B = 512            # batch rows
N = 96000          # samples per row
M = 8              # cores
R = B // M         # rows per core = 64
HALF = N // 2      # per-partition row-col span = 48000
SEED = 1500        # seed width == head section width
YSEC = 3000        # tail section width

C_T = 0.21         # fp16->fp8 boundary: T_b ~= C_T * decay_samples_b
T_BUCKET = 3000
T_FLOOR = 12000    # fewer tail sections beats the few extra fp16 bytes:
                   # compute (DVE+ACT) is the wall, the stream has slack
T_CAP = 24000

# schedule model (rel. to program start, us) used to order stores /
# balance the two tail producers; tuned against the HW trace.
# GpSimd is NOT a producer: a Pool tensor op on [128,3000] measured 43us
# AND stalled concurrent DVE ops to the same 43us (SBUF port thrash).
EST = {
    "dve_start": 3.5,    # loads landed (seedx 384KB via sync HWDGE)
    "seedy_op": 0.57,    # DVE fp16 1500-col op
    "x_op": 0.52,        # DVE fp16 1500-col section
    "dve_y": 1.70,       # DVE fp8 3000-col section
    "act_y": 2.80,       # ACT fp8 3000-col section
    "slow_start": 0.20,  # sem hop after seedY for ACT
}

_F32 = mybir.dt.float32
_F16 = mybir.dt.float16
_FP8 = mybir.dt.float8e4

_cached = {}


def _rates(decay):
    """f32 rate exactly as the reference computes it, plus f64 log."""
    d = np.asarray(decay, dtype=np.float32).reshape(B)
    decay_ms = np.float32(10.0) + np.float32(1990.0) * d
    ds = (decay_ms * np.float32(48000.0)) / np.float32(1000.0)
    rate = np.float32(1.0) - np.float32(6.91) / ds
    return ds.astype(np.float64), np.log(rate.astype(np.float64))


def _geometry(ds):
    """Shared-across-cores widths from the actual decay values."""
    order = np.argsort(-ds, kind="stable")          # slowest first
    ds_sorted = ds[order]
    octmax = ds_sorted[0::M][:R]                    # max ds of octet j
    T = np.ceil(C_T * octmax / T_BUCKET) * T_BUCKET
    T = np.clip(T, T_FLOOR, T_CAP).astype(np.int64)  # [R], multiple of 3000
    hw = T // 2                                     # head half-width, mult 1500
    tw = HALF - hw                                  # tail half-width, mult 1500
    hw_p = np.repeat(hw, 2)                         # [128] per partition
    tw_p = np.repeat(tw, 2)
    nx = int(hw.max()) // SEED                      # head sections (incl seed)
    ny = -(-int(tw.max()) // YSEC)                  # tail sections (ceil)
    # head store k (k=1..nx-1): partitions [0, PX_k)
    PX = [int(np.count_nonzero(hw_p > SEED * k)) for k in range(nx)]
    # tail store k (k=0..ny-1): partitions [PY_k, 128)
    PY = [int(np.argmax(tw_p > YSEC * k)) for k in range(ny)]
    return order, T, hw, tw, nx, ny, PX, PY


def _schedule(nx, ny):
    """Assign tail sections to engines (list scheduling by earliest
    completion) and produce the EDF store order.

    Returns (assign, store_order):
      assign[k] = (engine, op_idx_1based) for tail section k
      store_order = list of ("x", k) / ("y", k) sorted by est. readiness
    """
    seedy_done = EST["dve_start"] + 2 * EST["seedy_op"]
    x_done = [seedy_done + EST["x_op"] * k for k in range(1, nx)]  # X1..
    avail = {
        "v": seedy_done + EST["x_op"] * (nx - 1),
        "a": seedy_done + EST["slow_start"],
    }
    cost = {"v": EST["dve_y"], "a": EST["act_y"]}
    nops = {"v": 0, "a": 0}
    picks = []
    for _ in range(ny):
        eng = min(avail, key=lambda e: avail[e] + cost[e])
        avail[eng] += cost[eng]
        nops[eng] += 1
        picks.append((avail[eng], eng, nops[eng]))
    picks.sort()
    # tail section k is computed by the k-th completed pick
    assign = [(eng, idx) for (_t, eng, idx) in picks]
    stores = [(x_done[k - 1], ("x", k)) for k in range(1, nx)]
    stores += [(picks[k][0], ("y", k)) for k in range(ny)]
    stores.sort()
    return assign, [s for (_t, s) in stores]


def _build_bass(geom):
    order, T, hw, tw, nx, ny, PX, PY = geom
    hwmax, cy = nx * SEED, ny * YSEC
    KK = max(nx, 2 * ny - 1)   # coef cols 0..KK-1 hold r^(1500k)
    NC = KK + 2                # + the two seedY multipliers
    assign, store_order = _schedule(nx, ny)

    nc = bacc.Bacc("TRN2", target_bir_lowering=False, debug=False, num_devices=M)

    seedx_t = nc.dram_tensor("seedx", [128, SEED], _F16, kind="ExternalInput")
    coef_t = nc.dram_tensor("coef", [128, NC], _F32, kind="ExternalInput")
    outx_t = nc.dram_tensor("outx", [128, hwmax], _F16, kind="ExternalOutput")
    outy_t = nc.dram_tensor("outy", [128, cy], _FP8, kind="ExternalOutput")

    bigx = nc.alloc_sbuf_tensor("bigx", [128, hwmax], _F16)
    bigy = nc.alloc_sbuf_tensor("bigy", [128, cy], _FP8)
    seedy = nc.alloc_sbuf_tensor("seedy", [128, 2 * SEED], _F16)
    coef_s = nc.alloc_sbuf_tensor("coef_s", [128, NC], _F32)

    n_stores = 1 + (nx - 1) + ny   # D2D + head + tail

    with (
        nc.semaphore("l_sem") as l_sem,      # +16 seedx load done
        nc.semaphore("c_sem") as c_sem,      # +16 coef load done
        nc.semaphore("s_sem") as s_sem,      # +1 per seedY op
        nc.semaphore("v_sem") as v_sem,      # +1 per DVE section (X then Y)
        nc.semaphore("a_sem") as a_sem,      # +1 per ACT tail section
        nc.semaphore("d_sem") as d_sem,      # +16 per store
        nc.Block() as block,
    ):
        # DVE Y-op index -> v_sem threshold offset
        dve_y_rank = {}
        r = nx - 1
        for k, (eng, idx) in enumerate(assign):
            if eng == "v":
                dve_y_rank[k] = r + idx

        @block.sync
        def _(sync):
            # coef first: its (tiny) transfer and completion receipt hide
            # under the seedx transfer, so DVE wakes on seedx's receipt
            sync.dma_start(coef_s.ap(), coef_t.ap()).then_inc(c_sem, 16)
            sync.dma_start(bigx.ap()[:, 0:SEED], seedx_t.ap()).then_inc(l_sem, 16)
            # seed section -> output, DRAM->DRAM, in the load-latency window
            sync.dma_start(outx_t.ap()[:, 0:SEED], seedx_t.ap()).then_inc(d_sem, 16)
            for kind, k in store_order:
                if kind == "x":
                    sync.wait_ge(v_sem, k)
                    p = PX[k]
                    sync.dma_start(
                        outx_t.ap()[0:p, SEED * k : SEED * (k + 1)],
                        bigx.ap()[0:p, SEED * k : SEED * (k + 1)],
                    ).then_inc(d_sem, 16)
                else:
                    eng, idx = assign[k]
                    sem = v_sem if eng == "v" else a_sem
                    tgt = dve_y_rank[k] if eng == "v" else idx
                    sync.wait_ge(sem, tgt)
                    p = PY[k]
                    sync.dma_start(
                        outy_t.ap()[p:128, YSEC * k : YSEC * (k + 1)],
                        bigy.ap()[p:128, YSEC * k : YSEC * (k + 1)],
                    ).then_inc(d_sem, 16)
            sync.wait_ge(d_sem, 16 * n_stores)

        @block.vector
        def _(vector):
            vector.wait_ge(l_sem, 16)
            vector.wait_ge(c_sem, 16)
            for u in range(2):
                vector.tensor_scalar_mul(
                    seedy.ap()[:, u * SEED : (u + 1) * SEED],
                    bigx.ap()[:, 0:SEED],
                    coef_s.ap()[:, KK + u : KK + u + 1],
                ).then_inc(s_sem, 1)
            for k in range(1, nx):
                vector.tensor_scalar_mul(
                    bigx.ap()[:, SEED * k : SEED * (k + 1)],
                    bigx.ap()[:, 0:SEED],
                    coef_s.ap()[:, k : k + 1],
                ).then_inc(v_sem, 1)
            for k, (eng, _idx) in enumerate(assign):
                if eng == "v":
                    vector.tensor_scalar_mul(
                        bigy.ap()[:, YSEC * k : YSEC * (k + 1)],
                        seedy.ap(),
                        coef_s.ap()[:, 2 * k : 2 * k + 1],
                    ).then_inc(v_sem, 1)

        @block.scalar
        def _(scalar):
            scalar.wait_ge(s_sem, 2)
            for k, (eng, _idx) in enumerate(assign):
                if eng == "a":
                    scalar.activation(
                        bigy.ap()[:, YSEC * k : YSEC * (k + 1)],
                        seedy.ap(),
                        mybir.ActivationFunctionType.Copy,
                        scale=coef_s.ap()[:, 2 * k : 2 * k + 1],
                    ).then_inc(a_sem, 1)

    nc.finalize()
    return nc


def _host_precompute(geom, lnr):
    """Per-core seedx (fp16) and coef (f32) from fp64 host math."""
    order, T, hw, tw, nx, ny, PX, PY = geom
    KK = max(nx, 2 * ny - 1)
    NC = KK + 2
    q = np.arange(SEED, dtype=np.float64)
    h_p = np.tile(np.float64([0.0, 1.0]), R)        # [128]
    hw_p = np.repeat(hw, 2).astype(np.float64)
    tw_p = np.repeat(tw, 2).astype(np.float64)
    T_p = np.repeat(T, 2).astype(np.float64)
    in_maps = []
    for c in range(M):
        rows = order[c::M][:R]                      # local rank j -> row
        ln_p = np.repeat(lnr[rows], 2)              # [128]
        seedx = np.exp((h_p * hw_p)[:, None] * ln_p[:, None] + q[None, :] * ln_p[:, None])
        coef = np.empty((128, NC), dtype=np.float64)
        for k in range(KK):
            coef[:, k] = np.exp(SEED * k * ln_p)
        for u in range(2):
            coef[:, KK + u] = np.exp(
                (T_p + h_p * tw_p + SEED * u - h_p * hw_p) * ln_p
            )
        in_maps.append(
            {
                "seedx": seedx.astype(np.float16),
                "coef": coef.astype(np.float32),
            }
        )
    return in_maps


def _run(decay, **spmd_kwargs):
    ds, lnr = _rates(decay)
    key = ds.tobytes()
    if _cached.get("key") != key:
        geom = _geometry(ds)
        _cached.update(key=key, geom=geom, nc=_build_bass(geom))
    geom = _cached["geom"]
    order, T, hw, tw, nx, ny, PX, PY = geom
    in_maps = _host_precompute(geom, lnr)
    res = run_bass_kernel_spmd(_cached["nc"], in_maps, list(range(M)), **spmd_kwargs)
    out = np.empty((B, N), dtype=np.float32)
    for c in range(M):
        ox = np.asarray(res.results[c]["outx"]).astype(np.float32)
        oy = np.asarray(res.results[c]["outy"]).astype(np.float32)
        rows = order[c::M][:R]
        for j in range(R):
            b = rows[j]
            w, t = int(hw[j]), int(tw[j])
            out[b, 0:w] = ox[2 * j, 0:w]
            out[b, w : 2 * w] = ox[2 * j + 1, 0:w]
            out[b, 2 * w : 2 * w + t] = oy[2 * j, 0:t]
            out[b, 2 * w + t : N] = oy[2 * j + 1, 0:t]
    return out, res


def kernel(num_samples, decay):
    assert int(num_samples) == N, f"kernel compiled for {N} samples"
    out, _ = _run(decay)
    return out
